# revision 14
# baseline (speedup 1.0000x reference)
"""Trainium2 Bass kernel for the masked multi-head attention module.

Shapes (hardcoded): B=4, SQ=SK=1024, D=1024, H=16, DH=64.
Sharding over 8 cores: core c -> batch b=c//2, head-half hh=c%2 (8 heads),
output-column-half hh. Pairwise AllGather of ctx^T between cores (2b, 2b+1),
then each core computes a disjoint 512-column slice of the output.
"""

import os
import numpy as np

B, S, D, H, DH = 4, 1024, 1024, 16, 64
P = 128
NEG = -1.0e9

_CACHE = {}
LAST_RESULT = None


def _build_program():
    from concourse import bacc
    import concourse.bass as bass
    import concourse.tile as tile
    from concourse import mybir
    from concourse.masks import make_identity

    f32 = mybir.dt.float32
    f32r = mybir.dt.float32r
    Exp = mybir.ActivationFunctionType.Exp

    nc = bacc.Bacc("TRN2", target_bir_lowering=False, debug=False, num_devices=8)

    q_in = nc.dram_tensor("q_in", [S, D], f32, kind="ExternalInput")
    v_in = nc.dram_tensor("v_in", [S, D], f32, kind="ExternalInput")
    wq_d = nc.dram_tensor("wq", [D, 512], f32, kind="ExternalInput")
    wk_d = nc.dram_tensor("wk", [D, 512], f32, kind="ExternalInput")
    wv_d = nc.dram_tensor("wv", [D, 512], f32, kind="ExternalInput")
    wo_d = nc.dram_tensor("wo", [H * DH, 512], f32, kind="ExternalInput")
    bq_d = nc.dram_tensor("bq2", [P, 4], f32, kind="ExternalInput")
    bk_d = nc.dram_tensor("bk2", [P, 4], f32, kind="ExternalInput")
    bv_d = nc.dram_tensor("bv_row", [1, 512], f32, kind="ExternalInput")
    bo_d = nc.dram_tensor("bo_row", [1, 512], f32, kind="ExternalInput")
    vb_d = nc.dram_tensor("vbias", [P, 8], f32, kind="ExternalInput")
    qm_d = nc.dram_tensor("qm_rsh", [P, 16], f32, kind="ExternalInput")
    y_out = nc.dram_tensor("y_out", [S, 512], f32, kind="ExternalOutput")
    KDBG = os.environ.get("KDEBUG", "") == "1"
    if KDBG:
        dbg_qT = nc.dram_tensor("dbg_qT", [P, S], f32, kind="ExternalOutput")
        dbg_QTa = nc.dram_tensor("dbg_QTa", [P, S], f32, kind="ExternalOutput")
        dbg_QTb = nc.dram_tensor("dbg_QTb", [P, S], f32, kind="ExternalOutput")
        dbg_KT = nc.dram_tensor("dbg_KT", [P, S], f32, kind="ExternalOutput")
        dbg_V = nc.dram_tensor("dbg_V", [P, 520], f32, kind="ExternalOutput")
        dbg_ut = nc.dram_tensor("dbg_ut", [P, S], f32, kind="ExternalOutput")
        dbg_ctxA = nc.dram_tensor("dbg_ctxA", [65, S], f32, kind="ExternalOutput")
        dbg_st = nc.dram_tensor("dbg_st", [P, S], f32, kind="ExternalOutput")
        dbg_bc = nc.dram_tensor("dbg_bc", [P, S], f32, kind="ExternalOutput")

    groups = [[0, 1], [2, 3], [4, 5], [6, 7]]

    def bcast_ap(src_ap, nparts):
        # partition-broadcast read: iterate nparts partitions with stride 0
        return bass.AP(
            tensor=src_ap.tensor,
            offset=src_ap.offset,
            ap=[[0, nparts]] + list(src_ap.ap[1:]),
        )

    with tile.TileContext(nc) as tc:
        with (
            tc.tile_pool(name="A", bufs=26) as A,
            tc.tile_pool(name="W", bufs=26) as Wp,
            tc.tile_pool(name="VS", bufs=8) as VSp,
            tc.tile_pool(name="SM", bufs=1) as SM,
            tc.tile_pool(name="SM2", bufs=1) as SM2,
            tc.tile_pool(name="ps_sc", bufs=2, space="PSUM") as PSC,
            tc.tile_pool(name="ps_fx", bufs=2, space="PSUM") as PSF,
            tc.tile_pool(name="dram", bufs=4, space="DRAM") as DR,
        ):
            # ---- small constants ----
            ident = SM.tile([P, P], f32, tag="ident")
            make_identity(nc, ident[:])
            bq_sb = SM.tile([P, 4], f32, tag="bq")
            nc.sync.dma_start(out=bq_sb[:], in_=bq_d[:, :])
            bk_sb = SM.tile([P, 4], f32, tag="bk")
            nc.sync.dma_start(out=bk_sb[:], in_=bk_d[:, :])
            vb_sb = SM.tile([P, 8], f32, tag="vb")
            nc.sync.dma_start(out=vb_sb[:], in_=vb_d[:, :])
            qm_sb = SM.tile([P, 16], f32, tag="qm")
            nc.sync.dma_start(out=qm_sb[:], in_=qm_d[:, :])
            bv_bc = SM.tile([P, 512], f32, tag="bvb")
            nc.gpsimd.dma_start(out=bv_bc[:], in_=bcast_ap(bv_d[:, :], P))
            bo_bc = SM.tile([P, 512], f32, tag="bob")
            nc.gpsimd.dma_start(out=bo_bc[:], in_=bcast_ap(bo_d[:, :], P))

            # ---- load + transpose q and v ----
            def load_transpose(x_dram):
                x_sb = []
                for i in range(8):
                    t = A.tile([P, S], f32, tag="big")
                    nc.sync.dma_start(out=t[:], in_=x_dram[i * P:(i + 1) * P, :])
                    x_sb.append(t)
                xT = []
                for dt_i in range(8):
                    ps = PSF.tile([P, S], f32, tag="flex")
                    for qt_i in range(8):
                        nc.tensor.transpose(
                            ps[:, qt_i * P:(qt_i + 1) * P],
                            x_sb[qt_i][:, dt_i * P:(dt_i + 1) * P],
                            ident[:],
                        )
                    t = A.tile([P, S], f32, tag="big")
                    nc.vector.tensor_copy(t[:].bitcast(f32r), ps[:])
                    xT.append(t)
                return xT

            qT = load_transpose(q_in)
            vT = load_transpose(v_in)

            # ---- weights ----
            def load_w(w_dram):
                out = []
                for i in range(8):
                    t = Wp.tile([P, 512], f32, tag="w")
                    nc.sync.dma_start(out=t[:].bitcast(f32r), in_=w_dram[i * P:(i + 1) * P, :].bitcast(f32r))
                    out.append(t)
                return out

            wq_sb = load_w(wq_d)
            wk_sb = load_w(wk_d)
            wv_sb = load_w(wv_d)

            # ---- Q^T / K^T projections: out[ht] = [128 hd, 1024 q] ----
            def proj_T(w_sb, x_T, bias_sb):
                outs = []
                for ht in range(4):
                    ps = PSF.tile([P, S], f32, tag="flex")
                    for c in range(2):
                        for di in range(8):
                            nc.tensor.matmul(
                                ps[:, c * 512:(c + 1) * 512],
                                lhsT=w_sb[di][:, ht * P:(ht + 1) * P].bitcast(f32r),
                                rhs=x_T[di][:, c * 512:(c + 1) * 512].bitcast(f32r),
                                start=(di == 0),
                                stop=(di == 7),
                            )
                    t = A.tile([P, S], f32, tag="big")
                    nc.vector.tensor_scalar_add(t[:].bitcast(f32r), ps[:], bias_sb[:, ht:ht + 1])
                    outs.append(t)
                return outs

            # Q^T: per-head zero-padded tiles (rows 0:64 = even head, rows
            # 64:128 = odd head; other half zero) so scores contract K=128
            # without fp32r row-group tile_position (HW bug workaround).
            QTp = []
            for ht in range(4):
                ps = PSF.tile([P, S], f32, tag="flex")
                for c in range(2):
                    for di in range(8):
                        nc.tensor.matmul(
                            ps[:, c * 512:(c + 1) * 512],
                            lhsT=wq_sb[di][:, ht * P:(ht + 1) * P].bitcast(f32r),
                            rhs=qT[di][:, c * 512:(c + 1) * 512].bitcast(f32r),
                            start=(di == 0),
                            stop=(di == 7),
                        )
                tA = A.tile([P, S], f32, tag="big")
                nc.vector.memset(tA[64:128, :], 0.0)
                nc.vector.tensor_scalar_add(
                    tA[0:64, :].bitcast(f32r), ps[0:64, :], bq_sb[0:64, ht:ht + 1]
                )
                tB = A.tile([P, S], f32, tag="big")
                nc.vector.memset(tB[0:64, :], 0.0)
                nc.vector.tensor_scalar_add(
                    tB[64:128, :].bitcast(f32r), ps[64:128, :], bq_sb[64:128, ht:ht + 1]
                )
                QTp.extend([tA, tB])
            KT = proj_T(wk_sb, vT, bk_sb)

            # ---- V projection: Vst[kt] = [128 k, 8*65] (65th col = ones) ----
            Vst = []
            for kt in range(8):
                ps = PSF.tile([P, S], f32, tag="flex")
                for di in range(8):
                    nc.tensor.matmul(
                        ps[:, 0:512],
                        lhsT=vT[di][:, kt * P:(kt + 1) * P].bitcast(f32r),
                        rhs=wv_sb[di][:, :].bitcast(f32r),
                        start=(di == 0),
                        stop=(di == 7),
                    )
                t = VSp.tile([P, 8, 65], f32, tag="vst")
                nc.vector.memset(t[:], 1.0)
                nc.vector.tensor_add(
                    t[:, :, 0:64].bitcast(f32r),
                    ps[:, 0:512].rearrange("p (h d) -> p h d", h=8),
                    bv_bc[:, :].rearrange("p (h d) -> p h d", h=8),
                )
                Vst.append(t)

            if KDBG:
                nc.sync.dma_start(out=dbg_qT[:, :], in_=qT[0][:])
                nc.sync.dma_start(out=dbg_QTa[:, :], in_=QTp[0][:])
                nc.sync.dma_start(out=dbg_QTb[:, :], in_=QTp[1][:])
                nc.sync.dma_start(out=dbg_KT[:, :], in_=KT[0][:])
                nc.sync.dma_start(out=dbg_V[:, :], in_=Vst[0][:].rearrange("p h d -> p (h d)"))

            # ---- Wo load (used at the end) ----
            wo_sb = load_w(wo_d)

            # ---- attention per local head pair ----
            ctxT_full = [None] * 8
            for p in range(4):
                ctxA = PSF.tile([65, S], f32, tag="flex")
                ctxB = PSF.tile([65, S], f32, tag="flex")
                for kt in range(8):
                    for c in range(2):
                        sps = PSC.tile([P, S], f32, tag="sc")
                        nc.tensor.matmul(
                            sps[:, 0:512],
                            lhsT=KT[p][:, kt * P:(kt + 1) * P].bitcast(f32r),
                            rhs=QTp[2 * p][:, c * 512:(c + 1) * 512].bitcast(f32r),
                            start=True,
                            stop=True,
                        )
                        nc.tensor.matmul(
                            sps[:, 512:1024],
                            lhsT=KT[p][:, kt * P:(kt + 1) * P].bitcast(f32r),
                            rhs=QTp[2 * p + 1][:, c * 512:(c + 1) * 512].bitcast(f32r),
                            start=True,
                            stop=True,
                        )
                        ut = A.tile([P, S], f32, tag="big")
                        nc.scalar.activation(
                            ut[:].bitcast(f32r), sps[:], Exp, bias=vb_sb[:, kt:kt + 1], scale=1.0
                        )
                        if KDBG and p == 0 and kt == 0 and c == 0:
                            nc.sync.dma_start(out=dbg_ut[:, :], in_=ut[:])
                        nc.tensor.matmul(
                            ctxA[:, c * 512:(c + 1) * 512],
                            lhsT=Vst[kt][:, 2 * p, :].bitcast(f32r),
                            rhs=ut[:, 0:512].bitcast(f32r),
                            start=(kt == 0),
                            stop=(kt == 7),
                        )
                        nc.tensor.matmul(
                            ctxB[:, c * 512:(c + 1) * 512],
                            lhsT=Vst[kt][:, 2 * p + 1, :].bitcast(f32r),
                            rhs=ut[:, 512:1024].bitcast(f32r),
                            start=(kt == 0),
                            stop=(kt == 7),
                        )

                # normalization: r = q_mask / sums, broadcast over partitions
                sumA = SM2.tile([1, S], f32, tag="sumA")
                sumB = SM2.tile([1, S], f32, tag="sumB")
                nc.vector.tensor_copy(sumA[:], ctxA[64:65, :])
                nc.vector.tensor_copy(sumB[:], ctxB[64:65, :])
                rsh = SM2.tile([P, 16], f32, tag="rsh")
                nc.sync.dma_start(out=rsh[0:64, :], in_=sumA[:])
                nc.sync.dma_start(out=rsh[64:128, :], in_=sumB[:])
                rr = SM2.tile([P, 16], f32, tag="rr")
                nc.vector.reciprocal(rr[:], rsh[:])
                nc.vector.tensor_mul(rr[:], rr[:], qm_sb[:])
                rdram = DR.tile([2, S], f32, tag="rd")
                nc.sync.dma_start(out=rdram[0:1, :], in_=rr[0:64, :])
                nc.sync.dma_start(out=rdram[1:2, :], in_=rr[64:128, :])
                bc = A.tile([P, S], f32, tag="big")
                nc.gpsimd.dma_start(out=bc[0:64, :], in_=bcast_ap(rdram[0:1, :], 64))
                nc.gpsimd.dma_start(out=bc[64:128, :], in_=bcast_ap(rdram[1:2, :], 64))
                st = A.tile([P, S], f32, tag="big")
                nc.vector.tensor_mul(st[0:64, :].bitcast(f32r), ctxA[0:64, :], bc[0:64, :])
                nc.vector.tensor_mul(st[64:128, :].bitcast(f32r), ctxB[0:64, :], bc[64:128, :])

                if KDBG and p == 0:
                    dtmp = A.tile([65, S], f32, tag="big")
                    nc.vector.tensor_copy(dtmp[:], ctxA[:])
                    nc.sync.dma_start(out=dbg_ctxA[:, :], in_=dtmp[:])
                    nc.sync.dma_start(out=dbg_st[:, :], in_=st[:])
                    nc.sync.dma_start(out=dbg_bc[:, :], in_=bc[:])

                # pairwise exchange of ctx^T
                cin = DR.tile([P, S], f32, tag="ccin")
                nc.sync.dma_start(out=cin[:], in_=st[:])
                cout = DR.tile([2, P, S], f32, tag="ccout")
                import concourse.mybir as mybir_

                nc.gpsimd.collective_compute(
                    "AllGather",
                    mybir_.AluOpType.bypass,
                    replica_groups=groups,
                    ins=[cin[:].opt()],
                    outs=[cout[:].opt()],
                )
                ta = A.tile([P, S], f32, tag="big")
                nc.sync.dma_start(out=ta[:].bitcast(f32r), in_=cout[0, :, :].bitcast(f32r))
                tb = A.tile([P, S], f32, tag="big")
                nc.sync.dma_start(out=tb[:].bitcast(f32r), in_=cout[1, :, :].bitcast(f32r))
                ctxT_full[p] = ta
                ctxT_full[4 + p] = tb

            # ---- output projection (column slice) ----
            for qt in range(8):
                yp = PSF.tile([P, 512], f32, tag="flex")
                for ht in range(8):
                    nc.tensor.matmul(
                        yp[:, :],
                        lhsT=ctxT_full[ht][:, qt * P:(qt + 1) * P].bitcast(f32r),
                        rhs=wo_sb[ht][:, :].bitcast(f32r),
                        start=(ht == 0),
                        stop=(ht == 7),
                    )
                ysb = Wp.tile([P, 512], f32, tag="w")
                nc.vector.tensor_add(ysb[:], yp[:], bo_bc[:])
                nc.sync.dma_start(
                    out=y_out[qt * P:(qt + 1) * P, :], in_=ysb[:]
                )

    nc.compile()
    return nc


def _get_program():
    if "nc" not in _CACHE:
        _CACHE["nc"] = _build_program()
    return _CACHE["nc"]


def kernel(q, v, q_mask, v_mask, Wq, bq, Wk, bk, Wv, bv, Wo, bo):
    global LAST_RESULT
    from concourse.bass_utils import run_bass_kernel_spmd

    q = np.asarray(q, dtype=np.float32)
    v = np.asarray(v, dtype=np.float32)
    q_mask = np.asarray(q_mask)
    v_mask = np.asarray(v_mask)
    Wq = np.asarray(Wq, dtype=np.float32)
    Wk = np.asarray(Wk, dtype=np.float32)
    Wv = np.asarray(Wv, dtype=np.float32)
    Wo = np.asarray(Wo, dtype=np.float32)
    bq = np.asarray(bq, dtype=np.float32)
    bk = np.asarray(bk, dtype=np.float32)
    bv = np.asarray(bv, dtype=np.float32)
    bo = np.asarray(bo, dtype=np.float32)

    nc = _get_program()

    in_maps = []
    for c in range(8):
        b, hh = c // 2, c % 2
        hsl = slice(512 * hh, 512 * (hh + 1))
        vb = np.where(v_mask[b], 0.0, NEG).astype(np.float32)
        qm = q_mask[b].astype(np.float32)
        in_maps.append(
            {
                "q_in": np.ascontiguousarray(q[b]),
                "v_in": np.ascontiguousarray(v[b]),
                "wq": np.ascontiguousarray(Wq[:, hsl]),
                "wk": np.ascontiguousarray(Wk[:, hsl]),
                "wv": np.ascontiguousarray(Wv[:, hsl]),
                "wo": np.ascontiguousarray(Wo[:, hsl]),
                "bq2": np.ascontiguousarray(bq[hsl].reshape(4, P).T),
                "bk2": np.ascontiguousarray(bk[hsl].reshape(4, P).T),
                "bv_row": np.ascontiguousarray(bv[hsl].reshape(1, 512)),
                "bo_row": np.ascontiguousarray(bo[hsl].reshape(1, 512)),
                "vbias": np.ascontiguousarray(vb.reshape(8, P).T),
                "qm_rsh": np.ascontiguousarray(
                    np.tile(qm.reshape(64, 16), (2, 1))
                ),
            }
        )

    res = run_bass_kernel_spmd(
        nc,
        in_maps,
        core_ids=list(range(8)),
        tmpdir=os.environ.get("KERNEL_TRACE_DIR") or None,
    )
    LAST_RESULT = res

    out = np.empty((B, S, D), dtype=np.float32)
    for b in range(B):
        out[b, :, 0:512] = res.results[2 * b]["y_out"]
        out[b, :, 512:1024] = res.results[2 * b + 1]["y_out"]
    return out


# revision 15
# speedup vs baseline: 1.0300x; 1.0300x over previous
"""Trainium2 Bass kernel for the masked multi-head attention module.

Shapes (hardcoded): B=4, SQ=SK=1024, D=1024, H=16, DH=64.
Sharding over 8 cores: core c -> batch b=c//2, head-half hh=c%2 (8 heads),
output-column-half hh. Pairwise AllGather of ctx^T between cores (2b, 2b+1),
then each core computes a disjoint 512-column slice of the output.
"""

import os
import numpy as np

B, S, D, H, DH = 4, 1024, 1024, 16, 64
P = 128
NEG = -1.0e9

_CACHE = {}
LAST_RESULT = None


def _build_program():
    from concourse import bacc
    import concourse.bass as bass
    import concourse.tile as tile
    from concourse import mybir
    from concourse.masks import make_identity

    f32 = mybir.dt.float32
    f32r = mybir.dt.float32r
    Exp = mybir.ActivationFunctionType.Exp

    nc = bacc.Bacc("TRN2", target_bir_lowering=False, debug=False, num_devices=8)

    q_in = nc.dram_tensor("q_in", [S, D], f32, kind="ExternalInput")
    v_in = nc.dram_tensor("v_in", [S, D], f32, kind="ExternalInput")
    wq_d = nc.dram_tensor("wq", [D, 512], f32, kind="ExternalInput")
    wk_d = nc.dram_tensor("wk", [D, 512], f32, kind="ExternalInput")
    wv_d = nc.dram_tensor("wv", [D, 512], f32, kind="ExternalInput")
    wo_d = nc.dram_tensor("wo", [H * DH, 512], f32, kind="ExternalInput")
    bq_d = nc.dram_tensor("bq2", [P, 4], f32, kind="ExternalInput")
    bk_d = nc.dram_tensor("bk2", [P, 4], f32, kind="ExternalInput")
    bv_d = nc.dram_tensor("bv_row", [1, 512], f32, kind="ExternalInput")
    bo_d = nc.dram_tensor("bo_row", [1, 512], f32, kind="ExternalInput")
    vb_d = nc.dram_tensor("vbias", [P, 8], f32, kind="ExternalInput")
    qm_d = nc.dram_tensor("qm_rsh", [P, 16], f32, kind="ExternalInput")
    y_out = nc.dram_tensor("y_out", [S, 512], f32, kind="ExternalOutput")
    KDBG = os.environ.get("KDEBUG", "") == "1"
    if KDBG:
        dbg_qT = nc.dram_tensor("dbg_qT", [P, S], f32, kind="ExternalOutput")
        dbg_QTa = nc.dram_tensor("dbg_QTa", [P, S], f32, kind="ExternalOutput")
        dbg_QTb = nc.dram_tensor("dbg_QTb", [P, S], f32, kind="ExternalOutput")
        dbg_KT = nc.dram_tensor("dbg_KT", [P, S], f32, kind="ExternalOutput")
        dbg_V = nc.dram_tensor("dbg_V", [P, 520], f32, kind="ExternalOutput")
        dbg_ut = nc.dram_tensor("dbg_ut", [P, S], f32, kind="ExternalOutput")


    groups = [[0, 1], [2, 3], [4, 5], [6, 7]]

    def bcast_ap(src_ap, nparts):
        # partition-broadcast read: iterate nparts partitions with stride 0
        return bass.AP(
            tensor=src_ap.tensor,
            offset=src_ap.offset,
            ap=[[0, nparts]] + list(src_ap.ap[1:]),
        )

    with tile.TileContext(nc) as tc:
        with (
            tc.tile_pool(name="A", bufs=30) as A,
            tc.tile_pool(name="W", bufs=26) as Wp,
            tc.tile_pool(name="VS", bufs=8) as VSp,
            tc.tile_pool(name="SM", bufs=1) as SM,
            tc.tile_pool(name="SM2", bufs=1) as SM2,
            tc.tile_pool(name="ps_sc", bufs=2, space="PSUM") as PSC,
            tc.tile_pool(name="ps_fx", bufs=2, space="PSUM") as PSF,
            tc.tile_pool(name="dram", bufs=4, space="DRAM") as DR,
        ):
            # ---- small constants ----
            ident = SM.tile([P, P], f32, tag="ident")
            make_identity(nc, ident[:])
            bq_sb = SM.tile([P, 4], f32, tag="bq")
            nc.sync.dma_start(out=bq_sb[:], in_=bq_d[:, :])
            bk_sb = SM.tile([P, 4], f32, tag="bk")
            nc.sync.dma_start(out=bk_sb[:], in_=bk_d[:, :])
            vb_sb = SM.tile([P, 8], f32, tag="vb")
            nc.sync.dma_start(out=vb_sb[:], in_=vb_d[:, :])
            qm_sb = SM.tile([P, 16], f32, tag="qm")
            nc.sync.dma_start(out=qm_sb[:], in_=qm_d[:, :])
            bv_bc = SM.tile([P, 512], f32, tag="bvb")
            nc.gpsimd.dma_start(out=bv_bc[:], in_=bcast_ap(bv_d[:, :], P))
            bo_bc = SM.tile([P, 512], f32, tag="bob")
            nc.gpsimd.dma_start(out=bo_bc[:], in_=bcast_ap(bo_d[:, :], P))

            # ---- load + transpose q and v ----
            def load_transpose(x_dram):
                x_sb = []
                for i in range(8):
                    t = A.tile([P, S], f32, tag="big")
                    nc.sync.dma_start(out=t[:], in_=x_dram[i * P:(i + 1) * P, :])
                    x_sb.append(t)
                xT = []
                for dt_i in range(8):
                    ps = PSF.tile([P, S], f32, tag="flex")
                    for qt_i in range(8):
                        nc.tensor.transpose(
                            ps[:, qt_i * P:(qt_i + 1) * P],
                            x_sb[qt_i][:, dt_i * P:(dt_i + 1) * P],
                            ident[:],
                        )
                    t = A.tile([P, S], f32, tag="big")
                    nc.vector.tensor_copy(t[:].bitcast(f32r), ps[:])
                    xT.append(t)
                return xT

            qT = load_transpose(q_in)
            vT = load_transpose(v_in)

            # ---- weights ----
            def load_w(w_dram):
                out = []
                for i in range(8):
                    t = Wp.tile([P, 512], f32, tag="w")
                    nc.sync.dma_start(out=t[:].bitcast(f32r), in_=w_dram[i * P:(i + 1) * P, :].bitcast(f32r))
                    out.append(t)
                return out

            wq_sb = load_w(wq_d)
            wk_sb = load_w(wk_d)
            wv_sb = load_w(wv_d)

            # ---- Q^T / K^T projections: out[ht] = [128 hd, 1024 q] ----
            def proj_T(w_sb, x_T, bias_sb):
                outs = []
                for ht in range(4):
                    ps = PSF.tile([P, S], f32, tag="flex")
                    for c in range(2):
                        for di in range(8):
                            nc.tensor.matmul(
                                ps[:, c * 512:(c + 1) * 512],
                                lhsT=w_sb[di][:, ht * P:(ht + 1) * P].bitcast(f32r),
                                rhs=x_T[di][:, c * 512:(c + 1) * 512].bitcast(f32r),
                                start=(di == 0),
                                stop=(di == 7),
                            )
                    t = A.tile([P, S], f32, tag="big")
                    nc.vector.tensor_scalar_add(t[:].bitcast(f32r), ps[:], bias_sb[:, ht:ht + 1])
                    outs.append(t)
                return outs

            # Q^T: per-head zero-padded tiles (rows 0:64 = even head, rows
            # 64:128 = odd head; other half zero) so scores contract K=128
            # without fp32r row-group tile_position (HW bug workaround).
            QTp = []
            for ht in range(4):
                ps = PSF.tile([P, S], f32, tag="flex")
                for c in range(2):
                    for di in range(8):
                        nc.tensor.matmul(
                            ps[:, c * 512:(c + 1) * 512],
                            lhsT=wq_sb[di][:, ht * P:(ht + 1) * P].bitcast(f32r),
                            rhs=qT[di][:, c * 512:(c + 1) * 512].bitcast(f32r),
                            start=(di == 0),
                            stop=(di == 7),
                        )
                tA = A.tile([P, S], f32, tag="big")
                nc.vector.memset(tA[64:128, :], 0.0)
                nc.vector.tensor_scalar_add(
                    tA[0:64, :].bitcast(f32r), ps[0:64, :], bq_sb[0:64, ht:ht + 1]
                )
                tB = A.tile([P, S], f32, tag="big")
                nc.vector.memset(tB[0:64, :], 0.0)
                nc.vector.tensor_scalar_add(
                    tB[64:128, :].bitcast(f32r), ps[64:128, :], bq_sb[64:128, ht:ht + 1]
                )
                QTp.extend([tA, tB])
            KT = proj_T(wk_sb, vT, bk_sb)

            # ---- V projection: Vst[kt] = [128 k, 8*65] (65th col = ones) ----
            Vst = []
            for kt in range(8):
                ps = PSF.tile([P, S], f32, tag="flex")
                for di in range(8):
                    nc.tensor.matmul(
                        ps[:, 0:512],
                        lhsT=vT[di][:, kt * P:(kt + 1) * P].bitcast(f32r),
                        rhs=wv_sb[di][:, :].bitcast(f32r),
                        start=(di == 0),
                        stop=(di == 7),
                    )
                t = VSp.tile([P, 8, 65], f32, tag="vst")
                nc.vector.memset(t[:], 1.0)
                nc.vector.tensor_add(
                    t[:, :, 0:64].bitcast(f32r),
                    ps[:, 0:512].rearrange("p (h d) -> p h d", h=8),
                    bv_bc[:, :].rearrange("p (h d) -> p h d", h=8),
                )
                Vst.append(t)

            if KDBG:
                nc.sync.dma_start(out=dbg_qT[:, :], in_=qT[0][:])
                nc.sync.dma_start(out=dbg_QTa[:, :], in_=QTp[0][:])
                nc.sync.dma_start(out=dbg_QTb[:, :], in_=QTp[1][:])
                nc.sync.dma_start(out=dbg_KT[:, :], in_=KT[0][:])
                nc.sync.dma_start(out=dbg_V[:, :], in_=Vst[0][:].rearrange("p h d -> p (h d)"))

            # ---- Wo load (used at the end) ----
            wo_sb = load_w(wo_d)

            # ---- attention per local head pair ----
            ctxT_full = [None] * 8
            for p in range(4):
                ctxA = PSF.tile([65, S], f32, tag="flex")
                ctxB = PSF.tile([65, S], f32, tag="flex")
                for kt in range(8):
                    for c in range(2):
                        sps = PSC.tile([P, S], f32, tag="sc")
                        nc.tensor.matmul(
                            sps[:, 0:512],
                            lhsT=KT[p][:, kt * P:(kt + 1) * P].bitcast(f32r),
                            rhs=QTp[2 * p][:, c * 512:(c + 1) * 512].bitcast(f32r),
                            start=True,
                            stop=True,
                        )
                        nc.tensor.matmul(
                            sps[:, 512:1024],
                            lhsT=KT[p][:, kt * P:(kt + 1) * P].bitcast(f32r),
                            rhs=QTp[2 * p + 1][:, c * 512:(c + 1) * 512].bitcast(f32r),
                            start=True,
                            stop=True,
                        )
                        ut = A.tile([P, S], f32, tag="big")
                        nc.scalar.activation(
                            ut[:].bitcast(f32r), sps[:], Exp, bias=vb_sb[:, kt:kt + 1], scale=1.0
                        )
                        if KDBG and p == 0 and kt == 0 and c == 0:
                            nc.sync.dma_start(out=dbg_ut[:, :], in_=ut[:])
                        nc.tensor.matmul(
                            ctxA[:, c * 512:(c + 1) * 512],
                            lhsT=Vst[kt][:, 2 * p, :].bitcast(f32r),
                            rhs=ut[:, 0:512].bitcast(f32r),
                            start=(kt == 0),
                            stop=(kt == 7),
                        )
                        nc.tensor.matmul(
                            ctxB[:, c * 512:(c + 1) * 512],
                            lhsT=Vst[kt][:, 2 * p + 1, :].bitcast(f32r),
                            rhs=ut[:, 512:1024].bitcast(f32r),
                            start=(kt == 0),
                            stop=(kt == 7),
                        )

                # copy ctx out of PSUM right away (frees the psum slots for
                # the next pair), normalization happens on the SBUF copy
                st = A.tile([P, S], f32, tag="big")
                sumA = SM2.tile([1, S], f32, tag="sumA")
                sumB = SM2.tile([1, S], f32, tag="sumB")
                nc.vector.tensor_copy(st[0:64, :], ctxA[0:64, :])
                nc.vector.tensor_copy(sumA[:], ctxA[64:65, :])
                nc.vector.tensor_copy(st[64:128, :], ctxB[0:64, :])
                nc.vector.tensor_copy(sumB[:], ctxB[64:65, :])
                # r = q_mask / sums, broadcast over partitions via DRAM
                rsh = SM2.tile([P, 16], f32, tag="rsh")
                nc.sync.dma_start(out=rsh[0:64, :], in_=sumA[:])
                nc.sync.dma_start(out=rsh[64:128, :], in_=sumB[:])
                rr = SM2.tile([P, 16], f32, tag="rr")
                nc.vector.reciprocal(rr[:], rsh[:])
                nc.vector.tensor_mul(rr[:], rr[:], qm_sb[:])
                rdram = DR.tile([2, S], f32, tag="rd")
                nc.sync.dma_start(out=rdram[0:1, :], in_=rr[0:64, :])
                nc.sync.dma_start(out=rdram[1:2, :], in_=rr[64:128, :])
                bc = A.tile([P, S], f32, tag="big")
                nc.gpsimd.dma_start(out=bc[0:64, :], in_=bcast_ap(rdram[0:1, :], 64))
                nc.gpsimd.dma_start(out=bc[64:128, :], in_=bcast_ap(rdram[1:2, :], 64))
                nc.vector.tensor_mul(st[:].bitcast(f32r), st[:], bc[:])


                # pairwise exchange of ctx^T
                cin = DR.tile([P, S], f32, tag="ccin")
                nc.sync.dma_start(out=cin[:], in_=st[:])
                cout = DR.tile([2, P, S], f32, tag="ccout")
                import concourse.mybir as mybir_

                nc.gpsimd.collective_compute(
                    "AllGather",
                    mybir_.AluOpType.bypass,
                    replica_groups=groups,
                    ins=[cin[:].opt()],
                    outs=[cout[:].opt()],
                )
                ta = A.tile([P, S], f32, tag="big")
                nc.sync.dma_start(out=ta[:].bitcast(f32r), in_=cout[0, :, :].bitcast(f32r))
                tb = A.tile([P, S], f32, tag="big")
                nc.sync.dma_start(out=tb[:].bitcast(f32r), in_=cout[1, :, :].bitcast(f32r))
                ctxT_full[p] = ta
                ctxT_full[4 + p] = tb

            # ---- output projection (column slice) ----
            HT_ORDER = [0, 4, 1, 5, 2, 6, 3, 7]
            for qt in range(8):
                yp = PSF.tile([P, 512], f32, tag="flex")
                for i, ht in enumerate(HT_ORDER):
                    nc.tensor.matmul(
                        yp[:, :],
                        lhsT=ctxT_full[ht][:, qt * P:(qt + 1) * P].bitcast(f32r),
                        rhs=wo_sb[ht][:, :].bitcast(f32r),
                        start=(i == 0),
                        stop=(i == 7),
                    )
                ysb = Wp.tile([P, 512], f32, tag="w")
                nc.vector.tensor_add(ysb[:], yp[:], bo_bc[:])
                nc.sync.dma_start(
                    out=y_out[qt * P:(qt + 1) * P, :], in_=ysb[:]
                )

    nc.compile()
    return nc


def _get_program():
    if "nc" not in _CACHE:
        _CACHE["nc"] = _build_program()
    return _CACHE["nc"]


def kernel(q, v, q_mask, v_mask, Wq, bq, Wk, bk, Wv, bv, Wo, bo):
    global LAST_RESULT
    from concourse.bass_utils import run_bass_kernel_spmd

    q = np.asarray(q, dtype=np.float32)
    v = np.asarray(v, dtype=np.float32)
    q_mask = np.asarray(q_mask)
    v_mask = np.asarray(v_mask)
    Wq = np.asarray(Wq, dtype=np.float32)
    Wk = np.asarray(Wk, dtype=np.float32)
    Wv = np.asarray(Wv, dtype=np.float32)
    Wo = np.asarray(Wo, dtype=np.float32)
    bq = np.asarray(bq, dtype=np.float32)
    bk = np.asarray(bk, dtype=np.float32)
    bv = np.asarray(bv, dtype=np.float32)
    bo = np.asarray(bo, dtype=np.float32)

    nc = _get_program()

    in_maps = []
    for c in range(8):
        b, hh = c // 2, c % 2
        hsl = slice(512 * hh, 512 * (hh + 1))
        vb = np.where(v_mask[b], 0.0, NEG).astype(np.float32)
        qm = q_mask[b].astype(np.float32)
        in_maps.append(
            {
                "q_in": np.ascontiguousarray(q[b]),
                "v_in": np.ascontiguousarray(v[b]),
                "wq": np.ascontiguousarray(Wq[:, hsl]),
                "wk": np.ascontiguousarray(Wk[:, hsl]),
                "wv": np.ascontiguousarray(Wv[:, hsl]),
                "wo": np.ascontiguousarray(Wo[:, hsl]),
                "bq2": np.ascontiguousarray(bq[hsl].reshape(4, P).T),
                "bk2": np.ascontiguousarray(bk[hsl].reshape(4, P).T),
                "bv_row": np.ascontiguousarray(bv[hsl].reshape(1, 512)),
                "bo_row": np.ascontiguousarray(bo[hsl].reshape(1, 512)),
                "vbias": np.ascontiguousarray(vb.reshape(8, P).T),
                "qm_rsh": np.ascontiguousarray(
                    np.tile(qm.reshape(64, 16), (2, 1))
                ),
            }
        )

    res = run_bass_kernel_spmd(
        nc,
        in_maps,
        core_ids=list(range(8)),
        tmpdir=os.environ.get("KERNEL_TRACE_DIR") or None,
    )
    LAST_RESULT = res

    out = np.empty((B, S, D), dtype=np.float32)
    for b in range(B):
        out[b, :, 0:512] = res.results[2 * b]["y_out"]
        out[b, :, 512:1024] = res.results[2 * b + 1]["y_out"]
    return out
